# revision 16
# baseline (speedup 1.0000x reference)
"""MC-CNN stereo cost-volume kernel for 8 Trainium2 NeuronCores.

Reference computation:
  - 5x (3x3 VALID conv + ReLU), 112 channels, on two 192x448 images
  - per disparity d in [0,31]: MLP(concat(ref_feat, sec_feat shifted by d))
    layers 224->384->384->384->1, ReLU x3 + sigmoid
  - output (182, 438, 32) = -score, NaN where x+d out of range

Strategy:
  - Shard feature rows (Hf=182 -> 8 slabs of 23, last padded) across cores;
    each core gets a 33-row input slab (conv halo). No collectives.
  - MLP layer 1 factorizes: feat @ fw1.T = rf.T @ fw1[:, :112].T  (A, disparity
    independent) + sf.T @ fw1[:, 112:].T shifted by d (B, shift = column
    re-index). A/B computed once per slab, staged in DRAM, streamed per row.
  - Per (row, d): h1 = relu(A + shift(B, d)) then 3 remaining layers as
    matmuls with K=384 split into 3 chunks of 128.
  - Matmuls run as float32r (full-rate fp32 PE mode; needs even moving dim,
    so widths are rounded up to even over 440-wide padded tiles).
  - Layer 4 (M=1) runs in bf16 with 4 disparities packed into PE column
    groups via tile_position -> 4x fewer PE column streams for that layer.
"""

import os
import sys

import numpy as np

if os.path.isdir("/opt/trn_rl_repo") and "/opt/trn_rl_repo" not in sys.path:
    sys.path.insert(0, "/opt/trn_rl_repo")

H, W = 192, 448
C = 112
NCORES = 8
RS = 23          # output feature rows per core (8*23 = 184 >= Hf=182)
HS = RS + 10     # input image rows per core slab
Hf, Wf = H - 10, W - 10   # 182, 438
WfP = Wf + 2     # padded width (even-N fp32r matmuls)
D = 32           # disparities 0..31
MCH = 3          # 384 / 128

_CACHE = {}
LAST_RESULTS = None


def _build_nc(rs=RS, dd=D, num_cores=NCORES):
    from contextlib import ExitStack

    import concourse.bass as bass
    import concourse.mybir as mybir
    import concourse.tile as tile
    from concourse import bacc

    f32 = mybir.dt.float32
    f32r = mybir.dt.float32r
    bf16 = mybir.dt.bfloat16
    AF = mybir.ActivationFunctionType
    ALU = mybir.AluOpType

    hs = rs + 10
    # per-layer conv output (rows, cols)
    conv_dims = [(rs + 8, 446), (rs + 6, 444), (rs + 4, 442), (rs + 2, 440), (rs, Wf)]

    nc = bacc.Bacc("TRN2", target_bir_lowering=False, debug=False,
                   num_devices=num_cores)

    ref_s = nc.dram_tensor("ref_s", [hs, W], f32r, kind="ExternalInput").ap()
    sec_s = nc.dram_tensor("sec_s", [hs, W], f32r, kind="ExternalInput").ap()
    w1r = nc.dram_tensor("w1r", [9, C], f32r, kind="ExternalInput").ap()
    wls = [nc.dram_tensor(f"w{l}t", [C, 9, C], f32r, kind="ExternalInput").ap()
           for l in (2, 3, 4, 5)]
    cbs = [nc.dram_tensor(f"cb{l}", [C, 1], f32, kind="ExternalInput").ap()
           for l in (1, 2, 3, 4, 5)]
    fw1r = nc.dram_tensor("fw1r", [C, 384], f32r, kind="ExternalInput").ap()
    fw1s = nc.dram_tensor("fw1s", [C, 384], f32r, kind="ExternalInput").ap()
    fw2t = nc.dram_tensor("fw2t", [384, 384], f32r, kind="ExternalInput").ap()
    fw3t = nc.dram_tensor("fw3t", [384, 384], f32r, kind="ExternalInput").ap()
    fw4t = nc.dram_tensor("fw4t", [384, 1], f32r, kind="ExternalInput").ap()
    fb1 = nc.dram_tensor("fb1", [384, 1], f32, kind="ExternalInput").ap()
    fb2 = nc.dram_tensor("fb2", [384, 1], f32, kind="ExternalInput").ap()
    fb3 = nc.dram_tensor("fb3", [384, 1], f32, kind="ExternalInput").ap()
    fb4 = nc.dram_tensor("fb4", [1, 1], f32, kind="ExternalInput").ap()
    out = nc.dram_tensor("out", [dd, rs, Wf], f32, kind="ExternalOutput").ap()

    with tile.TileContext(nc) as tc, ExitStack() as ctx:
        consts = ctx.enter_context(tc.tile_pool(name="consts", bufs=1))
        dram = ctx.enter_context(tc.tile_pool(name="dram", bufs=1, space="DRAM"))
        # PSUM pools span all phases: conv/A/L2 share ps2's banks, L3 has
        # its own, L4 two more -> 8 banks total
        ps2_p = ctx.enter_context(tc.tile_pool(name="ps2", bufs=3, space="PSUM"))
        ps3_p = ctx.enter_context(tc.tile_pool(name="ps3", bufs=3, space="PSUM"))
        ps4_p = ctx.enter_context(tc.tile_pool(name="ps4", bufs=2, space="PSUM"))
        # A/B row tiles: opened before feat so phase C row loads need not
        # wait for the conv pool's SBUF to be released
        rows_p = ctx.enter_context(tc.tile_pool(name="rows", bufs=2))
        osb_p = ctx.enter_context(tc.tile_pool(name="osb", bufs=4))

        # ---- load weights to SBUF ----
        w1r_sb = consts.tile([9, C], f32r, tag="w1r")
        nc.sync.dma_start(out=w1r_sb[:], in_=w1r[:])
        wl_sb = []
        for l, wl in zip((2, 3, 4, 5), wls):
            t = consts.tile([C, 9, C], f32r, tag=f"w{l}t")
            nc.sync.dma_start(out=t[:], in_=wl[:])
            wl_sb.append(t)
        cb_sb = consts.tile([C, 5], f32, tag="cb")
        for li, cb in enumerate(cbs):
            nc.sync.dma_start(out=cb_sb[:, li:li + 1], in_=cb[:])
        fw1r_sb = consts.tile([C, 384], f32r, tag="fw1r")
        nc.sync.dma_start(out=fw1r_sb[:], in_=fw1r[:])
        fw1s_sb = consts.tile([C, 384], f32r, tag="fw1s")
        nc.sync.dma_start(out=fw1s_sb[:], in_=fw1s[:])
        fw2t_sb = []
        fw3t_sb = []
        for k in range(MCH):
            t2 = consts.tile([128, 384], f32r, tag=f"fw2t{k}")
            nc.sync.dma_start(out=t2[:], in_=fw2t[k * 128:(k + 1) * 128, :])
            fw2t_sb.append(t2)
            t3 = consts.tile([128, 384], f32r, tag=f"fw3t{k}")
            nc.sync.dma_start(out=t3[:], in_=fw3t[k * 128:(k + 1) * 128, :])
            fw3t_sb.append(t3)
        fw4t_sb = []
        for k in range(MCH):
            t4 = consts.tile([128, 1], f32r, tag=f"fw4t{k}")
            nc.sync.dma_start(out=t4[:], in_=fw4t[k * 128:(k + 1) * 128, :])
            fw4t_sb.append(t4)
        fb1_sb = consts.tile([128, MCH], f32, tag="fb1")
        fb2_sb = consts.tile([128, MCH], f32, tag="fb2")
        fb3_sb = consts.tile([128, MCH], f32, tag="fb3")
        for m in range(MCH):
            nc.sync.dma_start(out=fb1_sb[:, m:m + 1], in_=fb1[m * 128:(m + 1) * 128, :])
            nc.sync.dma_start(out=fb2_sb[:, m:m + 1], in_=fb2[m * 128:(m + 1) * 128, :])
            nc.sync.dma_start(out=fb3_sb[:, m:m + 1], in_=fb3[m * 128:(m + 1) * 128, :])
        fb4_sb = consts.tile([128, 1], f32, tag="fb4")
        nc.sync.dma_start(
            out=fb4_sb[:],
            in_=bass.AP(tensor=fb4.tensor, offset=fb4.offset, ap=[[0, 128], [1, 1]]))

        a_dr = dram.tile([MCH, 128, rs, WfP], f32, tag="a_dr")
        b_dr = dram.tile([MCH, 128, rs, WfP], f32, tag="b_dr")

        # ---- phase B: convs + A/B build (per image) ----
        with tc.tile_pool(name="feat", bufs=1) as feat, \
             tc.tile_pool(name="imp", bufs=3) as imp, \
             tc.tile_pool(name="abuf", bufs=4) as abuf:
            for img_ap, fw1x, dest, add_bias in (
                (ref_s, fw1r_sb, a_dr, True),
                (sec_s, fw1s_sb, b_dr, False),
            ):
                # layer 1: im2col in 8-row chunks (K=9); each chunk is
                # built by 3 strided DMAs (partition dim = dx, rows x cols)
                h1r, w1c = conv_dims[0]
                cur = feat.tile([C, h1r, w1c], f32r, tag="ping")
                for y0 in range(0, h1r, 8):
                    yn = min(8, h1r - y0)
                    imr = imp.tile([9, 8, w1c], f32r, tag="imrow", bufs=2)
                    for dy in range(3):
                        src = bass.AP(tensor=img_ap.tensor,
                                      offset=img_ap.offset + (y0 + dy) * W,
                                      ap=[[1, 3], [W, yn], [1, w1c]])
                        nc.sync.dma_start(out=imr[3 * dy:3 * dy + 3, :yn, :], in_=src)
                    for yy in range(yn):
                        y = y0 + yy
                        ps = ps2_p.tile([C, w1c], f32, tag="mm")
                        nc.tensor.matmul(ps[:], lhsT=w1r_sb[:], rhs=imr[:, yy, :],
                                         start=True, stop=True)
                        nc.scalar.activation(out=cur[:, y, :], in_=ps[:], func=AF.Relu,
                                             bias=cb_sb[:, 0:1])
                # layers 2..4: 9 accumulating matmuls per output row
                for li, (ho, wo) in enumerate(conv_dims[1:4]):
                    nxt = feat.tile([C, ho, wo], f32r,
                                    tag=("pong" if li % 2 == 0 else "ping"))
                    wsb = wl_sb[li]
                    for y in range(ho):
                        ps = ps2_p.tile([C, wo], f32, tag="mm")
                        for t9 in range(9):
                            dy, dx = divmod(t9, 3)
                            nc.tensor.matmul(ps[:], lhsT=wsb[:, t9, :],
                                             rhs=cur[:, y + dy, dx:dx + wo],
                                             start=(t9 == 0), stop=(t9 == 8))
                        nc.scalar.activation(out=nxt[:, y, :], in_=ps[:], func=AF.Relu,
                                             bias=cb_sb[:, li + 1:li + 2])
                    cur = nxt
                # layer 5 + A/B build interleaved per row: f5 lives only as a
                # per-row tile, so `ping` frees after layer 4 and the next
                # image's conv overlaps the A/B build.
                for y in range(rs):
                    ps5 = ps3_p.tile([C, Wf], f32, tag="mm")
                    for t9 in range(9):
                        dy, dx = divmod(t9, 3)
                        nc.tensor.matmul(ps5[:], lhsT=wl_sb[3][:, t9, :],
                                         rhs=cur[:, y + dy, dx:dx + Wf],
                                         start=(t9 == 0), stop=(t9 == 8))
                    f5r = imp.tile([C, Wf], f32r, tag="f5row", bufs=4)
                    nc.scalar.activation(out=f5r[:], in_=ps5[:], func=AF.Relu,
                                         bias=cb_sb[:, 4:5])
                    for m in range(MCH):
                        ps = ps2_p.tile([128, Wf], f32, tag="mm")
                        nc.tensor.matmul(
                            ps[:], lhsT=fw1x[:, m * 128:(m + 1) * 128],
                            rhs=f5r[:], start=True, stop=True)
                        bt = abuf.tile([128, WfP], f32, tag="ab")
                        nc.vector.memset(bt[:, Wf:], 0.0)
                        if (y * MCH + m) % 2 == 0:
                            nc.vector.tensor_scalar(
                                out=bt[:, :Wf], in0=ps[:],
                                scalar1=(fb1_sb[:, m:m + 1] if add_bias else 0.0),
                                scalar2=None, op0=ALU.add)
                        else:
                            nc.scalar.activation(
                                out=bt[:, :Wf], in_=ps[:], func=AF.Identity,
                                bias=(fb1_sb[:, m:m + 1] if add_bias else 0.0))
                        nc.sync.dma_start(out=dest[m, :, y, :], in_=bt[:])

        # ---- phase C: per (row, disparity-quad) MLP ----
        with tc.tile_pool(name="hbuf", bufs=2) as hbuf, \
             tc.tile_pool(name="h3p", bufs=6) as h3p:
            for y in range(rs):
                ar = []
                br = []
                for m in range(MCH):
                    at = rows_p.tile([128, WfP], f32, tag=f"ar{m}")
                    nc.sync.dma_start(out=at[:], in_=a_dr[m, :, y, :])
                    ar.append(at)
                    bt = rows_p.tile([128, WfP], f32, tag=f"br{m}")
                    nc.sync.dma_start(out=bt[:], in_=b_dr[m, :, y, :])
                    br.append(bt)
                for dq in range(0, dd, 4):
                    quad = list(range(dq, min(dq + 4, dd)))
                    h3q = []
                    for d in quad:
                        nv = Wf - d
                        nv2 = nv + (nv & 1)
                        h1 = []
                        for m in range(MCH):
                            h1p = hbuf.tile([128, WfP], f32, tag=f"h1p{m}")
                            nc.gpsimd.tensor_tensor(
                                out=h1p[:, :nv2], in0=ar[m][:, :nv2],
                                in1=br[m][:, d:d + nv2], op=ALU.add)
                            h1t = hbuf.tile([128, WfP], f32r, tag=f"h1{m}")
                            nc.vector.tensor_relu(out=h1t[:, :nv2], in_=h1p[:, :nv2])
                            h1.append(h1t)
                        h2 = []
                        for mo in range(MCH):
                            ps = ps2_p.tile([128, WfP], f32, tag="mm")
                            for k in range(MCH):
                                nc.tensor.matmul(
                                    ps[:, :nv2],
                                    lhsT=fw2t_sb[k][:, mo * 128:(mo + 1) * 128],
                                    rhs=h1[k][:, :nv2],
                                    start=(k == 0), stop=(k == MCH - 1))
                            h2t = hbuf.tile([128, WfP], f32r, tag=f"h2{mo}")
                            if mo == 0:
                                nc.scalar.activation(
                                    out=h2t[:, :nv2], in_=ps[:, :nv2], func=AF.Relu,
                                    bias=fb2_sb[:, mo:mo + 1])
                            else:
                                nc.vector.tensor_scalar(
                                    out=h2t[:, :nv2], in0=ps[:, :nv2],
                                    scalar1=fb2_sb[:, mo:mo + 1],
                                    scalar2=0.0, op0=ALU.add, op1=ALU.max)
                            h2.append(h2t)
                        h3 = []
                        for mo in range(MCH):
                            ps = ps3_p.tile([128, WfP], f32, tag="mm")
                            for k in range(MCH):
                                nc.tensor.matmul(
                                    ps[:, :nv2],
                                    lhsT=fw3t_sb[k][:, mo * 128:(mo + 1) * 128],
                                    rhs=h2[k][:, :nv2],
                                    start=(k == 0), stop=(k == MCH - 1))
                            h3t = h3p.tile([128, WfP], f32r, tag=f"h3{mo}")
                            if mo == 0:
                                nc.vector.tensor_scalar(
                                    out=h3t[:, :nv2], in0=ps[:, :nv2],
                                    scalar1=fb3_sb[:, mo:mo + 1],
                                    scalar2=0.0, op0=ALU.add, op1=ALU.max)
                            else:
                                nc.scalar.activation(
                                    out=h3t[:, :nv2], in_=ps[:, :nv2], func=AF.Relu,
                                    bias=fb3_sb[:, mo:mo + 1])
                            h3.append(h3t)
                        h3q.append((d, nv, nv2, h3))
                    # layer 4: per-disparity M=1 matmul
                    for j, (d, nv, nv2, h3) in enumerate(h3q):
                        ps4 = ps4_p.tile([1, WfP], f32, tag="ps4")
                        for k in range(MCH):
                            nc.tensor.matmul(
                                ps4[:, :nv2], lhsT=fw4t_sb[k][:],
                                rhs=h3[k][:, :nv2],
                                start=(k == 0), stop=(k == MCH - 1))
                        ot = osb_p.tile([1, WfP], f32, tag="ot")
                        nc.scalar.activation(out=ot[:, :nv2], in_=ps4[:, :nv2],
                                             func=AF.Sigmoid, bias=fb4_sb[0:1, :])
                        nc.sync.dma_start(out=out[d, y, 0:nv], in_=ot[:, 0:nv])

    nc.compile()
    return nc


def _get_nc(rs=RS, dd=D, num_cores=NCORES):
    key = (rs, dd, num_cores)
    if key not in _CACHE:
        _CACHE[key] = _build_nc(rs, dd, num_cores)
    return _CACHE[key]


def _prep_weights(w1, b1, w2, b2, w3, b3, w4, b4, w5, b5,
                  fw1, fb1, fw2, fb2, fw3, fb3, fw4, fb4):
    import ml_dtypes
    f = np.float32
    wm = {
        "w1r": np.ascontiguousarray(np.asarray(w1, f).reshape(C, 9).T),
        "cb1": np.asarray(b1, f).reshape(C, 1),
        "cb2": np.asarray(b2, f).reshape(C, 1),
        "cb3": np.asarray(b3, f).reshape(C, 1),
        "cb4": np.asarray(b4, f).reshape(C, 1),
        "cb5": np.asarray(b5, f).reshape(C, 1),
        "fw1r": np.ascontiguousarray(np.asarray(fw1, f)[:, :C].T),
        "fw1s": np.ascontiguousarray(np.asarray(fw1, f)[:, C:].T),
        "fw2t": np.ascontiguousarray(np.asarray(fw2, f).T),
        "fw3t": np.ascontiguousarray(np.asarray(fw3, f).T),
        "fw4t": np.ascontiguousarray(np.asarray(fw4, f).T),
        "fb1": np.asarray(fb1, f).reshape(384, 1),
        "fb2": np.asarray(fb2, f).reshape(384, 1),
        "fb3": np.asarray(fb3, f).reshape(384, 1),
        "fb4": np.asarray(fb4, f).reshape(1, 1),
    }
    for l, wl in ((2, w2), (3, w3), (4, w4), (5, w5)):
        wm[f"w{l}t"] = np.ascontiguousarray(
            np.asarray(wl, f).transpose(1, 2, 3, 0).reshape(C, 9, C))
    return wm


def kernel(ref, sec, w1, b1, w2, b2, w3, b3, w4, b4, w5, b5,
           fw1, fb1, fw2, fb2, fw3, fb3, fw4, fb4, disp_min, disp_max):
    global LAST_RESULTS
    from concourse.bass_utils import run_bass_kernel_spmd

    assert int(disp_min) == 0 and int(disp_max) == 31, \
        f"kernel hardcodes disparities 0..31, got {disp_min}..{disp_max}"

    ref = np.asarray(ref, np.float32)
    sec = np.asarray(sec, np.float32)
    wm = _prep_weights(w1, b1, w2, b2, w3, b3, w4, b4, w5, b5,
                       fw1, fb1, fw2, fb2, fw3, fb3, fw4, fb4)

    pad_rows = NCORES * RS + 10
    refp = np.zeros((pad_rows, W), np.float32)
    secp = np.zeros((pad_rows, W), np.float32)
    refp[:H] = ref
    secp[:H] = sec

    in_maps = []
    for i in range(NCORES):
        r0 = i * RS
        m = dict(wm)
        m["ref_s"] = np.ascontiguousarray(refp[r0:r0 + HS])
        m["sec_s"] = np.ascontiguousarray(secp[r0:r0 + HS])
        in_maps.append(m)

    nc = _get_nc()
    res = run_bass_kernel_spmd(nc, in_maps, core_ids=list(range(NCORES)))
    LAST_RESULTS = res

    # per-core (D, RS, Wf) -> (Hf, Wf, D)
    full = np.concatenate(
        [r["out"].transpose(1, 2, 0) for r in res.results], axis=0)[:Hf]
    cv = -full
    xs = np.arange(Wf)[None, :, None]
    ds_ = np.arange(D)[None, None, :]
    invalid = (xs + ds_) >= Wf
    cv = np.where(invalid, np.float32(np.nan), cv).astype(np.float32)
    return cv
